# revision 2
# baseline (speedup 1.0000x reference)
"""BesselKAN layer kernel v2 for Trainium2 (8 NeuronCores, data-parallel batch).

reference math:
    t  = tanh(x)                                  # [B, I]
    b0 = 1; b1 = t + 1; b2 = 3t*b1 + b0; b3 = 5t*b2 + b1
    y[b,o] = sum_{i,d} b_d[b,i] * W[i,o,d]        # W = bessel_coeffs [I, O, 4]
           = colsum(W0)[o] + sum_{d=1..3} (b_d @ W_d)[b,o]

Key layout choices (cost model charges matmuls by OUTPUT free size only):
  * output computed TRANSPOSED: yT[o, b] = W_d_chunk.T @ u_d, so the moving
    operand is the basis tensor and the bias is a per-partition [o, 1] vector
    folded into the ScalarE copy-out (activation Identity with AP bias).
  * x is host-pre-transposed (xt = x.T slice per core) so no PE transposes.
  * W chunks are used as f32r stationaries straight from DMA -- no casts,
    no C-combination (original Bessel bases b1,b2,b3 multiply raw planes,
    and each plane's colsum rides its own matmul: only W0's colsum remains,
    computed with ~free [o,2]-shaped f32r matmuls vs an all-ones moving pair).
  * basis prep in fp32 split across ACT/DVE/Pool:
      ACT : t = tanh(x);  s2 = (t + 0.5)^2
      DVE : u1 = t + 1;   u2 = 3*s2 + 0.25 (== 1+3t+3t^2);  q = 5t*u2
      Pool: u3 = q + u1   (== 1+6t+15t^2+15t^3)
  * W host-permuted to [g][i0][d][k][o] so one DMA delivers a full
    128-wide output-column group (all planes, all k) contiguously.
"""

import sys

if "/opt/trn_rl_repo" not in sys.path:
    sys.path.insert(0, "/opt/trn_rl_repo")

from contextlib import ExitStack

import numpy as np

import concourse.bass as bass
import concourse.tile as tile
from concourse import bacc, mybir
from concourse._compat import with_exitstack

P = 128
N_CORES = 8
B_FULL = 8192
I_DIM = 1024
O_DIM = 1024
NDEG = 4

FP32 = mybir.dt.float32
F32R = mybir.dt.float32r

DEFAULT_CFG = dict(
    n_w_early=3,   # W columns streamed inside the x window
    w_split=4,     # DMAs per W column-group (per-plane granularity)
    d2_lag=0,      # per-column stagger of k-driven leads
    d3_lag=3,      # in-column stagger of plane-3 behind plane-1
    y_defer=2,     # copyout deferral among the lead columns
    tail_y_defer=0,  # copyout deferral in the steady column phase
    u3_engine="dve",  # engine for the final u3 add
    x0_split=1,      # first x piece DMA split
    mm0_fine=1,      # first matmul step N-split
    w0_fine=False,   # split col0 d1 part in k-halves
    co_slots=(2, 5), # unused in continuous schedule (kept for sweeps)
    y_last_split=2,  # final copyout ACT/DVE parallel split
    pobufs=6, wbufs=4, xbufs=4, tbufs=4, ybufs=3, sbufs=3,
)



@with_exitstack
def _body(ctx: ExitStack, tc: "tile.TileContext", y_d, x_d, w_d,
          b_loc, i_dim, o_dim, cfg=None):
    cfg = {**DEFAULT_CFG, **(cfg or {})}
    nc = tc.nc
    KI = i_dim // P          # contraction tiles
    OCH = o_dim // P         # output column groups (yT partition groups)
    NB = min(512, b_loc)     # psum free size (one bank fp32)
    BCH = b_loc // NB        # batch column chunks
    XS = BCH                 # x pieces match psum batch halves
    XW = NB                  # x DMA piece width

    singles = ctx.enter_context(tc.tile_pool(name="singles", bufs=1))
    xpool = ctx.enter_context(tc.tile_pool(name="xpool", bufs=cfg["xbufs"]))
    tpool = ctx.enter_context(tc.tile_pool(name="tpool", bufs=cfg["tbufs"]))
    spool = ctx.enter_context(tc.tile_pool(name="spool", bufs=cfg["sbufs"]))
    wpool = ctx.enter_context(tc.tile_pool(name="wpool", bufs=cfg["wbufs"]))
    ypool = ctx.enter_context(tc.tile_pool(name="ypool", bufs=cfg["ybufs"]))
    psum_o = ctx.enter_context(
        tc.tile_pool(name="psum_o", bufs=cfg["pobufs"], space="PSUM"))
    psum_b = ctx.enter_context(tc.tile_pool(name="psum_b", bufs=1, space="PSUM"))

    # persistent basis tensors [i_part, ki, b]
    # written as f32r so the engines round to the PE's fp32r grid (the BIR
    # verifier rejects fp32-written data consumed by an fp32r matmul)
    u1 = singles.tile([P, KI, b_loc], F32R, name="u1")
    u2 = singles.tile([P, KI, b_loc], F32R, name="u2")
    u3 = singles.tile([P, KI, b_loc], F32R, name="u3")
    ones2 = singles.tile([P, 2], FP32, name="ones2")
    half = singles.tile([P, 1], FP32, name="half")
    bias_sb = singles.tile([P, OCH], FP32, name="bias_sb")
    bias_ps = psum_b.tile([P, 2 * OCH], FP32, name="bias_ps")
    consts_emitted = [False]

    def emit_consts():
        if not consts_emitted[0]:
            consts_emitted[0] = True
            nc.vector.memset(ones2[:], 1.0)
            nc.vector.memset(half[:], 0.5)

    # ---- u production (per ki, per x-piece) ------------------------------
    def emit_x(ki, s, split=1, after_first=None):
        bsl0 = s * XW
        xr = xpool.tile([P, XW], FP32, tag="xr")
        t = tpool.tile([P, XW], FP32, tag="t")
        s2 = spool.tile([P, XW], FP32, tag="s2")
        q = spool.tile([P, XW], FP32, tag="q")
        step = XW // split
        for j in range(split):
            jsl = slice(j * step, (j + 1) * step)
            bsl = slice(bsl0 + j * step, bsl0 + (j + 1) * step)
            nc.sync.dma_start(out=xr[:, jsl], in_=x_d[ki * P:(ki + 1) * P, bsl])
            if j == 0 and after_first is not None:
                after_first()
            emit_consts()
            nc.scalar.activation(out=t[:, jsl], in_=xr[:, jsl],
                                 func=mybir.ActivationFunctionType.Tanh)
            nc.scalar.activation(out=s2[:, jsl], in_=t[:, jsl],
                                 func=mybir.ActivationFunctionType.Square,
                                 bias=half[:], scale=1.0)
            nc.vector.tensor_scalar(out=u1[:, ki, bsl], in0=t[:, jsl],
                                    scalar1=1.0, scalar2=None,
                                    op0=mybir.AluOpType.add)
            nc.vector.tensor_scalar(out=u2[:, ki, bsl], in0=s2[:, jsl],
                                    scalar1=3.0, scalar2=0.25,
                                    op0=mybir.AluOpType.mult,
                                    op1=mybir.AluOpType.add)
            nc.vector.scalar_tensor_tensor(out=q[:, jsl], in0=t[:, jsl],
                                           scalar=5.0, in1=u2[:, ki, bsl],
                                           op0=mybir.AluOpType.mult,
                                           op1=mybir.AluOpType.mult)
            eng = nc.vector if cfg["u3_engine"] == "dve" else nc.gpsimd
            eng.tensor_tensor(out=u3[:, ki, bsl], in0=q[:, jsl],
                              in1=u1[:, ki, bsl], op=mybir.AluOpType.add)

    us = (u1, u2, u3)

    # matmul emission: per column group g and batch half c, software-
    # pipelined over k (plane d lags by cfg lag) so the PE FIFO never
    # head-blocks on late u2/u3 tiles.  advance(upto) emits steps <= upto,
    # letting the leading columns interleave with x-piece emission in valid
    # program order (u[k] written before any step reading it).
    planes_in = set()   # (g, dram plane index) whose DMA has been emitted

    class ColHalf:
        def __init__(self, g, wg, c, fine=1):
            self.g, self.wg, self.c = g, wg, c
            self.fine = fine
            self.lags = (0, cfg["d2_lag"], cfg["d3_lag"])
            self.po = psum_o.tile([P, NB], FP32, tag="po", name=f"po{g}_{c}")
            self.started = False
            self.n_done = 0
            self.kd = [0, 0, 0]

        def advance(self, upto):
            # emit (plane d, contraction tile k) matmuls whose W part DMA is
            # already emitted and whose k is within the staggered window.
            total = 3 * KI
            c = self.c
            for d in range(3):
                if (self.g, d + 1) not in planes_in:
                    continue
                hi = min(upto - self.lags[d], KI - 1)
                while self.kd[d] <= hi:
                    k = self.kd[d]
                    pieces = self.fine if (d == 0 and k == 0) else 1
                    sub = NB // pieces
                    for j in range(pieces):
                        jsl = slice(c * NB + j * sub, c * NB + (j + 1) * sub)
                        nc.tensor.matmul(
                            self.po[:, j * sub:(j + 1) * sub],
                            self.wg[:, d + 1, k, :],
                            us[d][:, k, jsl],
                            start=not self.started,
                            stop=self.n_done == total - 1 and j == pieces - 1,
                        )
                        self.started = True
                    self.n_done += 1
                    self.kd[d] += 1

        def finish(self):
            self.advance(KI + max(self.lags))
            assert self.n_done == 3 * KI, (self.g, self.c, self.n_done)
            return self.po

    def emit_copyout(g, c, po, split=1):
        if c == 0:
            nc.vector.tensor_copy(out=bias_sb[:, g:g + 1],
                                  in_=bias_ps[:, 2 * g:2 * g + 1])
        yo = ypool.tile([P, NB], FP32, tag="yo", name=f"yo{g}_{c}")
        if split == 1:
            nc.scalar.activation(out=yo[:], in_=po[:],
                                 func=mybir.ActivationFunctionType.Identity,
                                 bias=bias_sb[:, g:g + 1], scale=1.0)
            nc.scalar.dma_start(out=y_d[g * P:(g + 1) * P,
                                        c * NB:(c + 1) * NB], in_=yo[:])
            return
        # final group: halves on ACT and DVE in parallel to shorten the
        # post-last-matmul chain (copyout + HWDGE + DGE + transfer + sem)
        h = NB // 2
        nc.scalar.activation(out=yo[:, :h], in_=po[:, :h],
                             func=mybir.ActivationFunctionType.Identity,
                             bias=bias_sb[:, g:g + 1], scale=1.0)
        nc.scalar.dma_start(out=y_d[g * P:(g + 1) * P, c * NB:c * NB + h],
                            in_=yo[:, :h])
        nc.vector.tensor_scalar(out=yo[:, h:], in0=po[:, h:],
                                scalar1=bias_sb[:, g:g + 1], scalar2=None,
                                op0=mybir.AluOpType.add)
        nc.vector.dma_start(out=y_d[g * P:(g + 1) * P, c * NB + h:(c + 1) * NB],
                            in_=yo[:, h:])

    # ---- schedule --------------------------------------------------------
    # x pieces go batch-half-major (all s=0 pieces, then s=1): each column's
    # half-0 psum group is fully executable after only 1/XS of the x stream.
    # Per phase: n_w_early fresh W columns stream in, parts (plane order
    # d1,d0,d2,d3) spread between the x pieces; bias matmuls batch right
    # after the d0 part.  Resident columns lead k-driven on u[:, k, s];
    # fresh columns emit half 0 whole once their parts are in, then join
    # the k-driven leads for half s.  Copy-outs flush at fixed slots so
    # PSUM never exceeds the 8 banks and ACT waits resolve before dispatch.
    wgs = {}
    halves = {}
    pending_co = []
    retired = set()

    def get_half(g, c):
        if (g, c) not in halves:
            fine = cfg["mm0_fine"] if (g == 0 and c == 0) else 1
            halves[(g, c)] = ColHalf(g, wgs[g], c, fine=fine)
        return halves[(g, c)]

    def retire(g, c):
        if (g, c) in retired:
            return
        retired.add((g, c))
        get_half(g, c).finish()
        pending_co.append((g, c))

    def flush_co(limit, split=1):
        while pending_co and len(pending_co) > limit:
            g, c = pending_co.pop(0)
            emit_copyout(g, c, halves[(g, c)].po,
                         split=(split if not pending_co else 1))

    PLANE_ORDER = (1, 2, 3, 0)

    def emit_w_part(g, p, ksl=slice(0, KI)):
        d = PLANE_ORDER[p]
        nc.sync.dma_start(out=wgs[g][:, d, ksl, :],
                          in_=w_d[g, :, d, ksl, :])
        if ksl.stop == KI:
            planes_in.add((g, d))

    def emit_bias(g):
        for k in range(KI):
            nc.tensor.matmul(
                bias_ps[:, 2 * g:2 * g + 2],
                wgs[g][:, 0, k, :],
                ones2.bitcast(F32R)[:],
                start=k == 0,
                stop=k == KI - 1,
            )

    n_lead = max(1, min(cfg["n_w_early"], OCH))
    # continuous k-major x stream: both batch halves of each k arrive
    # back-to-back; the first n_lead W columns stream alongside, parts
    # round-robin across columns in plane-priority order (d1 first, bias
    # plane d0 last).  Every live (column, half) advances each slot with a
    # per-column stagger, so the PE FIFO tracks data arrival.
    lead = list(range(n_lead))
    for g in lead:
        wgs[g] = wpool.tile([P, NDEG, KI, P], F32R, tag="wg", name=f"wg{g}")
    parts = []
    for p in range(NDEG):
        for g in lead:
            if p == 0 and g == lead[0] and cfg["w0_fine"]:
                parts.append((g, 0, slice(0, KI // 2)))
                parts.append((g, 0, slice(KI // 2, KI)))
            else:
                parts.append((g, p, slice(0, KI)))
    nslots = KI * XS
    pi = 0
    d1_in = []
    pending_bias = []
    slot = 0

    def first_part():
        g, p, ksl = parts[0]
        emit_w_part(g, p, ksl)
        d1_in.append(g)

    for k in range(KI):
        for s in range(XS):
            if slot == 0:
                emit_x(k, s, split=cfg["x0_split"], after_first=first_part)
                pi = 1
            else:
                emit_x(k, s)
            slot += 1
            for g in pending_bias:
                emit_bias(g)
            pending_bias = []
            want = slot * len(parts) // nslots
            while pi < want:
                g, p, ksl = parts[pi]
                emit_w_part(g, p, ksl)
                pi += 1
                if p == 0:
                    d1_in.append(g)
                elif p == NDEG - 1:
                    pending_bias.append(g)
            for li, g in enumerate(d1_in):
                for c in range(s + 1):
                    get_half(g, c).advance(k - cfg["d2_lag"] * li - (s - c))
    for g in pending_bias:
        emit_bias(g)
    done_w = n_lead
    for li, g in enumerate(lead):
        for c in range(BCH):
            retire(g, c)
        flush_co(cfg["y_defer"])
    flush_co(cfg["y_defer"])
    # remaining columns
    for g in range(done_w, OCH):
        wgs[g] = wpool.tile([P, NDEG, KI, P], F32R, tag="wg", name=f"wg{g}")
        for p in range(NDEG):
            emit_w_part(g, p)
        emit_bias(g)
        for c in range(BCH):
            retire(g, c)
        flush_co(cfg["tail_y_defer"])
    flush_co(0, split=cfg["y_last_split"])

    assert len(retired) == OCH * BCH, (len(retired), OCH, BCH)


def build_nc(b_loc=B_FULL // N_CORES, i_dim=I_DIM, o_dim=O_DIM,
             n_cores=N_CORES, cfg=None):
    nc = bacc.Bacc("TRN2", target_bir_lowering=False, debug=False,
                   num_devices=n_cores)
    ki = i_dim // P
    och = o_dim // P
    x_d = nc.dram_tensor("x", [i_dim, b_loc], FP32, kind="ExternalInput").ap()
    w_d = nc.dram_tensor("w", [och, P, NDEG, ki, P], F32R,
                         kind="ExternalInput").ap()
    y_d = nc.dram_tensor("y", [o_dim, b_loc], FP32, kind="ExternalOutput").ap()
    with tile.TileContext(nc) as tc:
        _body(tc, y_d, x_d, w_d, b_loc, i_dim, o_dim, cfg=cfg)
    nc.compile()
    return nc


def permute_w(w, i_dim=I_DIM, o_dim=O_DIM):
    """[I, O, D] -> [och, 128(i0), D, ki, 128(o)] with i = ki*128 + i0."""
    ki = i_dim // P
    och = o_dim // P
    v = w.reshape(ki, P, och, P, NDEG)          # [ki, i0, g, o, d]
    v = v.transpose(2, 1, 4, 0, 3)              # [g, i0, d, ki, o]
    return np.ascontiguousarray(v)


_NC_CACHE = {}


def _get_nc(cfg=None):
    key = "full"
    if key not in _NC_CACHE:
        _NC_CACHE[key] = build_nc(cfg=cfg)
    return _NC_CACHE[key]


def run_spmd(x, bessel_coeffs, trace=False, cfg=None, **kwargs):
    from concourse.bass_utils import run_bass_kernel_spmd

    nc = _get_nc(cfg)
    x = np.asarray(x, dtype=np.float32)
    w = np.asarray(bessel_coeffs, dtype=np.float32)
    b_loc = x.shape[0] // N_CORES
    wp = permute_w(w)
    in_maps = []
    for c in range(N_CORES):
        xt = np.ascontiguousarray(x[c * b_loc:(c + 1) * b_loc].T)
        in_maps.append({"x": xt, "w": wp})
    res = run_bass_kernel_spmd(nc, in_maps, core_ids=list(range(N_CORES)),
                               trace=trace, **kwargs)
    y = np.concatenate([r["y"].T for r in res.results], axis=0)
    return np.ascontiguousarray(y), res


def kernel(x, bessel_coeffs):
    y, _ = run_spmd(x, bessel_coeffs)
    return y.astype(np.float32)


def _ref_np(x, w):
    t = np.tanh(np.asarray(x, dtype=np.float64))
    w = np.asarray(w, dtype=np.float64)
    basis = [np.ones_like(t), t + 1.0]
    for i in range(2, NDEG):
        basis.append((2 * i - 1) * t * basis[i - 1] + basis[i - 2])
    bz = np.stack(basis, axis=-1)
    return np.einsum("bid,iod->bo", bz, w)


def _selftest_sim(b_loc=512, i_dim=256, o_dim=512):
    from concourse.bass_interp import CoreSim

    nc = build_nc(b_loc=b_loc, i_dim=i_dim, o_dim=o_dim, n_cores=1)
    rng = np.random.default_rng(0)
    x = rng.standard_normal((b_loc, i_dim)).astype(np.float32)
    w = (rng.standard_normal((i_dim, o_dim, NDEG)) / (i_dim * NDEG)).astype(
        np.float32)
    sim = CoreSim(nc)
    sim.tensor("x")[:] = np.ascontiguousarray(x.T)
    sim.tensor("w")[:] = permute_w(w, i_dim=i_dim, o_dim=o_dim)
    sim.simulate()
    y = np.array(sim.tensor("y")).T
    ref = _ref_np(x, w)
    scale = np.abs(ref).max()
    err = np.abs(y - ref).max() / scale
    print(f"sim scale={scale:.4g} max_abs_rel_err={err:.4g}")
    assert err < 2e-2, err
    print("SIM OK")


def _timeline():
    from concourse.timeline_sim import TimelineSim

    nc = _get_nc()
    ts = TimelineSim(nc)
    sim_ns = ts.simulate()
    print(f"TimelineSim exec time: {sim_ns:.0f} ns")


if __name__ == "__main__":
    if "--sim" in sys.argv:
        _selftest_sim()
    if "--time" in sys.argv:
        _timeline()
